# revision 35
# baseline (speedup 1.0000x reference)
"""Trainium2 Bass kernel for a dense transformer block (B=4, T=2048, D=1024, H=16).

Sharding: 8 cores = 4 batches x 2 head-halves.  Each core computes LN1
(folded into weights host-side), Q/K/V for its 8 heads over all 2048
tokens, causal attention in feature-major layout (denominator via a
ones-column appended to V), then a row-parallel Wo partial that is
pairwise ReduceScattered so that each core ends up with a 1024-token
half for LN2 + MLP.

v2 vs baseline:
  - all matmuls in bf16 (same PE rate as fp32r at 512 free, half DMA/SBUF)
  - rstd folded into xs once (no token-major rstd DRAM bounce)
  - attention weights resident in SBUF, loaded once
  - batched softmax denominator (one reciprocal per q-tile)
  - ReduceScatter issued early and overlapped with compute
  - x2 kept in SBUF; xTo preloaded; MLP weights streamed per half
"""

import os
import sys

for _p in ("/opt/trn_rl_repo", "/root/.axon_site/_ro/trn_rl_repo"):
    if os.path.isdir(_p) and _p not in sys.path:
        sys.path.append(_p)

import numpy as np

import concourse.bass as bass
import concourse.tile as tile
from concourse import bacc, mybir
from concourse.bass_utils import run_bass_kernel_spmd

AF = mybir.ActivationFunctionType
ALU = mybir.AluOpType
FP32 = mybir.dt.float32
FP32R = mybir.dt.float32r
BF16 = mybir.dt.bfloat16
FP8 = mybir.dt.float8e4
WSCALE = 64.0  # fp8 weight scale for W1/W2

B, T, D, H = 4, 2048, 1024, 16
HD = D // H          # 64
DFF = 4 * D          # 4096
P = 128
DK = D // P          # 8   D k-tiles
NT = T // 512        # 4   512-token tiles
HC = H // 2          # 8   local heads per core
DQ = HC * HD         # 512 local qkv width
NOT = DQ // P        # 4   local head-pair tiles
FFT = DFF // P       # 32  DFF tiles
TOWN = T // 2        # 1024 own tokens after ReduceScatter
EPS = 1e-5
SCALE = 1.0 / 8.0    # 1/sqrt(HD)


def build_program(sim_mode=False):
    nc = bacc.Bacc("TRN2", target_bir_lowering=False, debug=False)

    # ---- DRAM I/O ----
    xT = nc.dram_tensor("xT", [D, T], BF16, kind="ExternalInput")
    xTo = nc.dram_tensor("xTo", [D, TOWN], FP32R, kind="ExternalInput")
    wqk = nc.dram_tensor("wqk", [P, 2, NOT, DK, P], BF16, kind="ExternalInput")
    wv = nc.dram_tensor("wv", [P, DK, DQ], BF16, kind="ExternalInput")
    wo = nc.dram_tensor("wo", [P, NOT, D], BF16, kind="ExternalInput")
    w1 = nc.dram_tensor("w1", [P, FFT, DK, P], BF16, kind="ExternalInput")
    w2 = nc.dram_tensor("w2", [P, DK, FFT, P], BF16, kind="ExternalInput")
    cqk = nc.dram_tensor("cqk", [P, 2 * NOT], FP32, kind="ExternalInput")
    cvb = nc.dram_tensor("cvb", [P, DQ], FP32, kind="ExternalInput")
    bo = nc.dram_tensor("bo", [P, DK], FP32, kind="ExternalInput")
    c1 = nc.dram_tensor("c1", [P, FFT], FP32, kind="ExternalInput")
    b2 = nc.dram_tensor("b2", [P, DK], FP32, kind="ExternalInput")
    masks = nc.dram_tensor("masks", [P, 896], BF16, kind="ExternalInput")
    out = nc.dram_tensor("out", [DK, P, TOWN], FP32R, kind="ExternalOutput")

    xT_r = xT.rearrange("(k p) t -> p k t", p=P)
    xTo_r = xTo.rearrange("(k p) t -> p k t", p=P)
    out_r = out.rearrange("k p t -> p k t")

    with tile.TileContext(nc) as tc:
        with (
            tc.tile_pool(name="pers", bufs=1) as pers,
            tc.tile_pool(name="psum", bufs=1, space="PSUM") as psum,
            tc.tile_pool(name="dram", bufs=1, space="DRAM") as dram,
        ):
            # ---- persistent tiles ----
            wqk_sb = pers.tile([P, 2, NOT, DK, P], BF16)
            nc.gpsimd.dma_start(wqk_sb, wqk[:, :, :, :, :])
            wv_sb = pers.tile([P, DK, DQ], BF16)
            nc.gpsimd.dma_start(wv_sb, wv[:, :, :])
            wo_sb = pers.tile([P, NOT, D], BF16)
            nc.gpsimd.dma_start(wo_sb, wo[:, :, :])
            mask_sb = pers.tile([P, 896], BF16)
            nc.scalar.dma_start(mask_sb, masks[:, :])
            cqk_sb = pers.tile([P, 2 * NOT], FP32)
            nc.scalar.dma_start(cqk_sb, cqk[:, :])
            cvb_sb = pers.tile([P, DQ], FP32)
            nc.scalar.dma_start(cvb_sb, cvb[:, :])
            bo_sb = pers.tile([P, DK], FP32)
            nc.scalar.dma_start(bo_sb, bo[:, :])
            c1_sb = pers.tile([P, FFT], FP32)
            nc.scalar.dma_start(c1_sb, c1[:, :])
            b2_sb = pers.tile([P, DK], FP32)
            nc.scalar.dma_start(b2_sb, b2[:, :])
            x2_sb = pers.tile([P, DK, TOWN], FP32R)
            # prefill x2 with the residual x at start (no dependencies);
            # consume adds the attention partials in place
            nc.sync.dma_start(x2_sb, xTo_r[:, :, :])
            rsb2_sb = pers.tile([P, TOWN], FP32)
            rs2_row = pers.tile([1, TOWN], FP32)

            ones_bf = pers.tile([P, 1], BF16)
            nc.vector.memset(ones_bf, 1.0)
            ones_fr = pers.tile([P, 1], FP32R)
            nc.vector.memset(ones_fr.bitcast(FP32), 1.0)
            eps_sb = pers.tile([1, 1], FP32)
            nc.vector.memset(eps_sb, EPS)
            # warm the ACT tables (Sqrt/Exp/Gelu) so no mid-kernel table loads
            warm = pers.tile([1, 3], FP32)
            nc.scalar.activation(warm[:, 0:1], eps_sb, AF.Sqrt, bias=eps_sb)
            nc.scalar.activation(warm[:, 1:2], eps_sb, AF.Exp)
            nc.scalar.activation(warm[:, 2:3], eps_sb, AF.Gelu, bias=eps_sb)
            nc.scalar.activation(warm[:, 0:1], eps_sb, AF.Ln)

            # DRAM scratch for the ReduceScatter (bf16 partials)
            rs_in = [dram.tile([2, DK, P, 512], BF16, name=f"rsin{i}",
                               tag=f"rsin{i}") for i in range(2)]
            rs_out = [dram.tile([DK, P, 512], BF16, name=f"rsout{i}",
                                tag=f"rsout{i}") for i in range(2)]

            # ==== attention phases, software-pipelined over 512-token tiles ====
            with tc.tile_pool(name="att", bufs=1) as att:
                kT_sb = att.tile([P, NOT, T], BF16)       # [64*hb+d, pair, t]
                v_sb = att.tile([P, HC, T // P, HD + 1], BF16)
                for h in range(HC):
                    nc.vector.memset(v_sb[:, h, :, HD:HD + 1], 1.0)

                qcur_t = [None] * NT
                stats_t = [None] * NT  # (xt_t, xs_t, rsb)

                def emit_stats(tt):
                    """xT load + LN1 stats + rstd + xs for tile tt (one
                    iteration ahead of its consumer B(tt))."""
                    ts5 = slice(tt * 512, (tt + 1) * 512)
                    xt_t = att.tile([P, DK, 512], BF16, tag="xt", bufs=2,
                                    name=f"xt{tt}")
                    nc.sync.dma_start(xt_t, xT_r[:, :, ts5])
                    s_ps = psum.tile([1, 512], FP32, tag="st", bufs=2,
                                     name=f"sps{tt}")
                    q_ps = psum.tile([1, 512], FP32, tag="st", bufs=2,
                                     name=f"qps{tt}")
                    for kt in range(DK):
                        nc.tensor.matmul(s_ps, ones_bf, xt_t[:, kt, :],
                                         start=(kt == 0), stop=(kt == DK - 1))
                    for kt in range(DK):
                        xsq = att.tile([P, 512], BF16, tag="xsq", bufs=1,
                                       name=f"xsq{tt}")
                        nc.vector.tensor_mul(xsq, xt_t[:, kt, :], xt_t[:, kt, :])
                        nc.tensor.matmul(q_ps, ones_bf, xsq,
                                         start=(kt == 0), stop=(kt == DK - 1))
                    mu = att.tile([1, 512], FP32, tag="murow", bufs=1,
                                  name=f"mu{tt}")
                    row_a = att.tile([1, 512], FP32, tag="rowa", bufs=1,
                                     name=f"rowa{tt}")
                    row_b = att.tile([1, 512], FP32, tag="rowb", bufs=1,
                                     name=f"rowb{tt}")
                    rs_row = att.tile([1, 512], FP32, tag="rsrow", bufs=2,
                                      name=f"rsrow{tt}")
                    nc.vector.tensor_scalar(mu, s_ps, 1.0 / D, None, ALU.mult)
                    nc.vector.tensor_scalar(row_a, q_ps, 1.0 / D, None, ALU.mult)
                    nc.vector.tensor_mul(row_b, mu, mu)
                    nc.vector.tensor_sub(row_a, row_a, row_b)
                    nc.scalar.activation(row_b, row_a, AF.Sqrt, bias=eps_sb)
                    nc.vector.reciprocal(rs_row, row_b)
                    rsb = att.tile([P, 512], FP32, tag="rsb", bufs=2,
                                   name=f"rsb{tt}")
                    nc.gpsimd.partition_broadcast(rsb, rs_row)
                    # xs = x * rstd for the V path only; q/k matmuls run on
                    # raw xt and get rstd folded into their output scale.
                    xs_t = att.tile([P, DK, 512], BF16, tag="xs", bufs=2,
                                    name=f"xs{tt}")
                    for kt in range(DK):
                        nc.vector.tensor_mul(xs_t[:, kt, :], xt_t[:, kt, :], rsb)
                    stats_t[tt] = (xt_t, xs_t, rsb)

                for tt in range(NT + 1):
                    if tt == 0:
                        emit_stats(0)

                    if tt >= 1:
                        # ---- C: attention for q-tile qt = tt-1 ----
                        qt = tt - 1
                        qv = qcur_t[qt]
                        nkt = 4 * qt + 4
                        ysb = att.tile([P, NOT, 512], BF16, tag="ysb", bufs=2)
                        den8 = att.tile([8, 512], FP32, tag="den8", bufs=1)
                        rden8 = att.tile([8, 512], BF16, tag="rden8", bufs=1)
                        yc_t = [[None] * 2 for _ in range(NOT)]
                        for pt in range(NOT):
                            y_ps = [psum.tile([HD + 1, 512], FP32, tag="y", bufs=2,
                                              name=f"yps{hb}")
                                    for hb in range(2)]
                            for kt in range(nkt):
                                jband = kt - 4 * qt
                                pexp = []
                                for hb in range(2):
                                    hsl = slice(hb * HD, (hb + 1) * HD)
                                    s_ps2 = psum.tile([P, 512], FP32, tag="mm", bufs=4)
                                    nc.tensor.matmul(
                                        s_ps2,
                                        kT_sb[hsl, pt, kt * P:(kt + 1) * P],
                                        qv[hsl, pt, :], start=True, stop=True)
                                    pe = att.tile([P, 512], BF16, tag="pexp", bufs=3)
                                    nc.scalar.activation(pe, s_ps2, AF.Exp, scale=SCALE)
                                    if jband >= 0:
                                        moff = 384 - P * jband
                                        nc.vector.tensor_mul(
                                            pe, pe, mask_sb[:, moff:moff + 512])
                                    pexp.append(pe)
                                for hb in range(2):
                                    nc.tensor.matmul(
                                        y_ps[hb], v_sb[:, 2 * pt + hb, kt, :], pexp[hb],
                                        start=(kt == 0), stop=(kt == nkt - 1))
                            for hb in range(2):
                                j = 2 * pt + hb
                                # den row (psum partition 64) -> sbuf -> den8[j]
                                cpden = att.tile([HD + 1, 512], FP32, tag="cpden",
                                                 bufs=2)
                                nc.scalar.copy(cpden[HD:HD + 1, :],
                                               y_ps[hb][HD:HD + 1, :])
                                nc.sync.dma_start(den8[j:j + 1, :],
                                                  cpden[HD:HD + 1, :])
                                # unnormalized y out of psum (bf16)
                                yc = att.tile([HD, 512], BF16, tag="yc", bufs=8)
                                nc.vector.tensor_scalar(yc, y_ps[hb][0:HD, :],
                                                        1.0, None, ALU.mult)
                                yc_t[pt][hb] = yc
                        with nc.allow_low_precision(reason="bf16 1/den"):
                            nc.vector.reciprocal(rden8, den8)
                        for pt in range(NOT):
                            for hb in range(2):
                                j = 2 * pt + hb
                                stage = att.tile([1, 512], BF16, tag="stage", bufs=2)
                                nc.gpsimd.dma_start(stage, rden8[j:j + 1, :])
                                rb = att.tile([HD, 512], BF16, tag="rb", bufs=2)
                                nc.gpsimd.partition_broadcast(rb, stage)
                                if hb == 0:
                                    nc.vector.tensor_mul(ysb[0:HD, pt, :],
                                                         yc_t[pt][hb], rb)
                                else:
                                    yst = att.tile([HD, 512], BF16, tag="yst", bufs=2)
                                    nc.vector.tensor_mul(yst, yc_t[pt][hb], rb)
                                    nc.sync.dma_start(ysb[HD:2 * HD, pt, :], yst)
                        # ---- D: Wo partials for q-tile qt ----
                        for ot in range(DK):
                            pp = psum.tile([P, 512], FP32, tag="mm", bufs=4)
                            for pt in range(NOT):
                                nc.tensor.matmul(pp, wo_sb[:, pt, ot * P:(ot + 1) * P],
                                                 ysb[:, pt, :],
                                                 start=(pt == 0), stop=(pt == NOT - 1))
                            ast = att.tile([P, 512], BF16, tag="ast", bufs=2)
                            nc.scalar.copy(ast, pp)
                            nc.sync.dma_start(rs_in[qt // 2][qt % 2, ot], ast)
                        if qt == 1:
                            if sim_mode:
                                nc.sync.dma_start(rs_out[0][:, :, :], rs_in[0][0])
                            else:
                                nc.gpsimd.collective_compute(
                                    "ReduceScatter", ALU.add,
                                    replica_groups=[[0, 1], [2, 3], [4, 5], [6, 7]],
                                    ins=[rs_in[0].opt()], outs=[rs_out[0].opt()])

                    if tt < NT:
                        if tt + 1 < NT:
                            emit_stats(tt + 1)
                        # ---- B: q/k/v projections for tile tt ----
                        xt_t, xs_t, rsb = stats_t[tt]
                        ts5 = slice(tt * 512, (tt + 1) * 512)
                        qcur = att.tile([P, NOT, 512], BF16, tag="qcur", bufs=2)
                        qcur_t[tt] = qcur
                        for proj in range(2):  # 0=q, 1=k
                            for ot in range(NOT):
                                pp = psum.tile([P, 512], FP32, tag="mm", bufs=4)
                                for kt in range(DK):
                                    nc.tensor.matmul(
                                        pp, wqk_sb[:, proj, ot, kt, :], xt_t[:, kt, :],
                                        start=(kt == 0), stop=(kt == DK - 1))
                                dest = (qcur[:, ot, :] if proj == 0
                                        else kT_sb[:, ot, ts5])
                                tmp = att.tile([P, 512], BF16, tag="ptmp", bufs=2)
                                nc.vector.tensor_mul(tmp, pp, rsb)
                                nc.vector.tensor_scalar(
                                    dest, tmp,
                                    cqk_sb[:, proj * NOT + ot:proj * NOT + ot + 1],
                                    None, ALU.add)
                        for st in range(4):
                            pp = psum.tile([P, 512], FP32, tag="mm", bufs=4)
                            for kt in range(DK):
                                nc.tensor.matmul(
                                    pp, xs_t[:, kt, st * P:(st + 1) * P],
                                    wv_sb[:, kt, :],
                                    start=(kt == 0), stop=(kt == DK - 1))
                            nc.vector.tensor_tensor(
                                v_sb[:, :, tt * 4 + st, 0:HD],
                                pp.rearrange("p (h e) -> p h e", h=HC),
                                cvb_sb.rearrange("p (h e) -> p h e", h=HC), ALU.add)


            # RS for the second half: emitted after the attention pool closes
            # so the pool-teardown DMA-drain barrier does not chain consume(0)
            # behind this collective's transfers.
            if sim_mode:
                nc.sync.dma_start(rs_out[1][:, :, :], rs_in[1][0])
            else:
                nc.gpsimd.collective_compute(
                    "ReduceScatter", ALU.add,
                    replica_groups=[[0, 1], [2, 3], [4, 5], [6, 7]],
                    ins=[rs_in[1].opt()], outs=[rs_out[1].opt()])

            # ==== consume halves + MLP ====
            with tc.tile_pool(name="fg", bufs=1) as fg:
                for i in range(2):
                    io5 = slice(i * 512, (i + 1) * 512)
                    # ---- consume: x2 = RS partial + bo + x, LN2 stats ----
                    att_t = fg.tile([P, DK, 512], BF16, tag="att", bufs=2)
                    nc.sync.dma_start(att_t, rs_out[i].rearrange("k p t -> p k t"))
                    s2_ps = psum.tile([1, 512], FP32, tag="st", bufs=2)
                    q2_ps = psum.tile([1, 512], FP32, tag="st", bufs=2)
                    for kt in range(DK):
                        nc.vector.scalar_tensor_tensor(
                            x2_sb[:, kt, io5], att_t[:, kt, :], bo_sb[:, kt:kt + 1],
                            x2_sb[:, kt, io5], ALU.add, ALU.add)
                        nc.tensor.matmul(s2_ps, ones_fr, x2_sb[:, kt, io5],
                                         start=(kt == 0), stop=(kt == DK - 1))
                        xsq2 = fg.tile([P, 512], FP32R, tag="xsq2", bufs=2)
                        nc.vector.tensor_mul(xsq2, x2_sb[:, kt, io5],
                                             x2_sb[:, kt, io5])
                        nc.tensor.matmul(q2_ps, ones_fr, xsq2,
                                         start=(kt == 0), stop=(kt == DK - 1))
                    mu2 = fg.tile([1, 512], FP32, tag="mu2", bufs=2)
                    row2a = fg.tile([1, 512], FP32, tag="row2a", bufs=2)
                    row2b = fg.tile([1, 512], FP32, tag="row2b", bufs=2)
                    nc.vector.tensor_scalar(mu2, s2_ps, 1.0 / D, None, ALU.mult)
                    nc.vector.tensor_scalar(row2a, q2_ps, 1.0 / D, None, ALU.mult)
                    nc.vector.tensor_mul(row2b, mu2, mu2)
                    nc.vector.tensor_sub(row2a, row2a, row2b)
                    nc.scalar.activation(row2b, row2a, AF.Sqrt, bias=eps_sb)
                    nc.vector.reciprocal(rs2_row[0:1, io5], row2b)
                    nc.gpsimd.partition_broadcast(rsb2_sb[:, io5], rs2_row[0:1, io5])

                    # ---- MLP over this 512-token half ----
                    x2s_t = fg.tile([P, DK, 512], BF16, tag="x2s", bufs=2)
                    for kt in range(DK):
                        nc.vector.tensor_mul(x2s_t[:, kt, :], x2_sb[:, kt, io5],
                                             rsb2_sb[:, io5])
                    m_sb = fg.tile([P, FFT, 512], BF16, tag="m", bufs=1)
                    for fft in range(FFT):
                        w1b = fg.tile([P, DK, P], BF16, tag="w1b", bufs=4)
                        nc.gpsimd.dma_start(w1b, w1[:, fft, :, :])
                        pp = psum.tile([P, 512], FP32, tag="mm", bufs=4)
                        for kt in range(DK):
                            nc.tensor.matmul(pp, w1b[:, kt, :], x2s_t[:, kt, :],
                                             start=(kt == 0), stop=(kt == DK - 1))
                        nc.scalar.activation(m_sb[:, fft, :], pp, AF.Gelu,
                                             bias=c1_sb[:, fft:fft + 1])
                    for ot in range(DK):
                        w2b = fg.tile([P, FFT, P], BF16, tag="w2b", bufs=2)
                        nc.gpsimd.dma_start(w2b, w2[:, ot, :, :])
                        pp = psum.tile([P, 512], FP32, tag="mm", bufs=4)
                        for kk in range(FFT):
                            nc.tensor.matmul(pp, w2b[:, kk, :], m_sb[:, kk, :],
                                             start=(kk == 0), stop=(kk == FFT - 1))
                        ost = fg.tile([P, 512], FP32R, tag="ost", bufs=2)
                        nc.vector.scalar_tensor_tensor(
                            ost, pp, b2_sb[:, ot:ot + 1], x2_sb[:, ot, io5],
                            ALU.add, ALU.add)
                        nc.sync.dma_start(out_r[:, ot, io5], ost)

    nc.compile()
    return nc


_NC_CACHE = None


def _get_nc():
    global _NC_CACHE
    if _NC_CACHE is None:
        _NC_CACHE = build_program()
    return _NC_CACHE


def prep_in_maps(x, ln1_g, ln1_b, ln2_g, ln2_b, Wq, bq, Wk, bk, Wv, bv,
                 Wo, bo, W1, b1, W2, b2):
    import ml_dtypes
    bf = ml_dtypes.bfloat16
    f32 = np.float32
    x = np.asarray(x, f32)
    ln1_g, ln1_b = np.asarray(ln1_g, f32), np.asarray(ln1_b, f32)
    ln2_g, ln2_b = np.asarray(ln2_g, f32), np.asarray(ln2_b, f32)
    Wq, Wk, Wv, Wo = (np.asarray(a, f32) for a in (Wq, Wk, Wv, Wo))
    W1, W2 = np.asarray(W1, f32), np.asarray(W2, f32)
    bq, bk, bv, bo_, b1, b2_ = (np.asarray(a, f32) for a in (bq, bk, bv, bo, b1, b2))

    # fold LN gain AND the mean subtraction (a rank-1 correction) into W:
    # (x - mu) * g @ W  =  x @ (g*W - colsum(g*W)/D)
    Wqg = ln1_g[:, None] * Wq
    Wkg = ln1_g[:, None] * Wk
    Wvg = ln1_g[:, None] * Wv
    Wqg = Wqg - Wqg.sum(0, keepdims=True) / D
    Wkg = Wkg - Wkg.sum(0, keepdims=True) / D
    Wvg = Wvg - Wvg.sum(0, keepdims=True) / D
    cq_full = ln1_b @ Wq + bq
    ck_full = ln1_b @ Wk + bk
    cv_full = ln1_b @ Wv + bv
    W1g = ln2_g[:, None] * W1
    W1g = W1g - W1g.sum(0, keepdims=True) / D
    c1_full = ln2_b @ W1 + b1

    w1_t = np.ascontiguousarray(
        W1g.reshape(DK, P, FFT, P).transpose(1, 2, 0, 3)).astype(bf)  # [P,FFT,DK,P]
    w2_t = np.ascontiguousarray(
        W2.reshape(FFT, P, DK, P).transpose(1, 2, 0, 3)).astype(bf)   # [P,DK,FFT,P]
    c1_t = np.ascontiguousarray(c1_full.reshape(FFT, P).T)            # [P,FFT]
    b2_t = np.ascontiguousarray(b2_.reshape(DK, P).T)                 # [P,DK]
    bo_t = np.ascontiguousarray(bo_.reshape(DK, P).T)                 # [P,DK]

    kk = np.arange(P)[:, None]
    cc = np.arange(896)[None, :]
    mk = (kk + 384 <= cc).astype(bf)

    in_maps = []
    for c in range(8):
        b_idx, hh = c // 2, c % 2
        sl = slice(DQ * hh, DQ * hh + DQ)
        xT_c = np.ascontiguousarray(x[b_idx].T)
        wq_c, wk_c = Wqg[:, sl], Wkg[:, sl]
        # [P, 2, NOT, DK, P]: arr[p,proj,ot,kt,m] = W[kt*P+p, ot*P+m]
        wqk_t = np.ascontiguousarray(
            np.stack([w.reshape(DK, P, NOT, P) for w in (wq_c, wk_c)])
            .transpose(2, 0, 3, 1, 4)).astype(bf)
        cq_t = cq_full[sl].reshape(NOT, P).T                     # [P,NOT]
        ck_t = ck_full[sl].reshape(NOT, P).T
        # own tokens after RS: q-tile (hh) then q-tile (hh+2)
        own = np.concatenate([xT_c[:, hh * 512:(hh + 1) * 512],
                              xT_c[:, (hh + 2) * 512:(hh + 3) * 512]], axis=1)
        in_maps.append({
            "xT": xT_c.astype(bf),
            "xTo": np.ascontiguousarray(own),
            "wqk": wqk_t,
            "wv": np.ascontiguousarray(
                Wvg[:, sl].reshape(DK, P, DQ).transpose(1, 0, 2)).astype(bf),
            "wo": np.ascontiguousarray(
                Wo[sl, :].reshape(NOT, P, D).transpose(1, 0, 2)).astype(bf),
            "w1": w1_t,
            "w2": w2_t,
            "cqk": np.ascontiguousarray(np.concatenate([cq_t, ck_t], axis=1)),
            "cvb": np.broadcast_to(cv_full[sl][None, :], (P, DQ)).copy(),
            "bo": bo_t,
            "c1": c1_t,
            "b2": b2_t,
            "masks": mk,
        })
    return in_maps


def assemble_output(results):
    out = np.empty((B, T, D), np.float32)
    for c in range(8):
        b_idx, hh = c // 2, c % 2
        o = results[c]["out"].reshape(D, TOWN)
        for i in range(2):
            qt = 2 * i + hh
            out[b_idx, qt * 512:(qt + 1) * 512, :] = o[:, i * 512:(i + 1) * 512].T
    return out


def kernel(**inputs):
    nc = _get_nc()
    in_maps = prep_in_maps(**inputs)
    res = run_bass_kernel_spmd(nc, in_maps, list(range(8)))
    return assemble_output(res.results)
